# revision 8
# baseline (speedup 1.0000x reference)
"""Weighted-MSE loss kernel (nn_LossWithEuler) for 8 Trainium2 NeuronCores.

loss = mean(weight[b] * (inp[d,b] - label[d,b])^2)
  weight[b]  = attr_w[b] * angle_w[b]
  attr_w[b]  = sum_j (attribute[j,b]==1) * (sum(attribute_num)/attribute_num[j])
  angle_w[b] = sum_j (1 - cos(ea[j,b])) = sum_j 2*sin(ea[j,b]/2)^2

Sharding: batch axis B=131072 split across 8 cores (16384 b's each).

inp/label are host-cast to fp8-e4m3 (quantization error on the final mean
~1.6e-3 vs the 2e-2 gate; halves the HBM stream vs bf16) and delivered in
COLUMN layout: partition = d (rows 0..127), free = b.  The leftover 8
d-rows (128..135) are repacked into 1024 "tail" columns of 16 b-blocks
(8 d's each).  In this layout the d-reduction is a ones-matmul on the
otherwise-idle TensorEngine: 32 sliding-window A-matmuls (512 cols each)
+ 2 block-A tail matmuls accumulate sum_d diff^2 per b into ONE [32,512]
f32 PSUM tile (colsq[r,q] = b = r*512+q).  A PE warmup spin at kernel
start keeps the PE at its 2.4 GHz HAM clock.  Chunks routed to PE_SUB are
delivered pair-interleaved (partition 2k = inp d-row, 2k+1 = label d-row)
and subtracted BY the PE with a fixed +-1 stationary; ACT squares those
from PSUM.  Remaining chunks: DVE subtracts (fp8 TT runs 1x), ACT (+DVE
for the last chunks) squares.  GPSIMD is kept idle: concurrent GPSIMD
tensor ops slow DVE ops ~2.5x (SBUF contention, measured).  The per-b
weight is computed in the natural [128,128] layout (b = p*128+f) and
shuffled to the colsq layout by 4 PE selector matmuls (w'[r,128*beta+f] =
w_nat[4r+beta, f]).  Final: one fused STT (colsq * w' with free-dim
accumulate), a [32]->[1] ones matmul, one f32 DMA out per core; host sums
8 partials / (D*B).
"""

import os
import sys
import numpy as np

D = 136
B = 131072
N_CORES = 8
BS = B // N_CORES       # 16384 b's per core
P = 128
NTAIL = BS // 16        # 1024 tail columns (16 b's x 8 d's each)
NCH = 16                # main chunks of 1024 cols
CHW = 1024              # chunk width (2 PE windows of 512)

# data8 field offsets (fp8 bytes per partition)
ATTR_OFF = 0
ATTR_LEN = 6 * 128      # 768
ASUB_OFF = ATTR_OFF + ATTR_LEN          # [128, 64] +-1 pair-subtract matrix
CH_OFF = ASUB_OFF + 64                  # chunks: 17 x (i 1024 | l 1024)
F8 = CH_OFF + (NCH + 1) * 2 * CHW

# aux16 field offsets (bf16 elems per partition)
EA_OFF = 0              # [128, 384]
ANUM_OFF = 384          # [128, 6]
AWIN_OFF = 392          # [128, 63]  main sliding ones-window
ZT_OFF = 456            # [128, 48]  tail block window
ASEL_OFF = 504          # [128, 4*32] selector matrices
F16 = 640

# engine routing (chunk indices 0..15 main, 16 = tail)
GP_SUB = set()                  # gpsimd sub chunks (hurts DVE; keep empty)
PE_SUB = {1, 3, 5, 7}           # pair-interleaved chunks subtracted on PE
DVE_SQ = {12, 13, 14, 15, 16}   # square chunks on DVE (rest on ACT)
WARM_MM = 5                     # PE warmup matmuls (512 cols each)
DMA_GROUPS = [2, 3, 4, 4, 4]    # chunk DMAs batched to cut issue cost


def _routing():
    gp = GP_SUB
    pe = PE_SUB
    dq = DVE_SQ
    warm = WARM_MM
    if os.environ.get("K_GP_SUB") is not None:
        gp = {int(x) for x in os.environ["K_GP_SUB"].split(",") if x != ""}
    if os.environ.get("K_PE_SUB") is not None:
        pe = {int(x) for x in os.environ["K_PE_SUB"].split(",") if x != ""}
    if os.environ.get("K_DVE_SQ") is not None:
        dq = {int(x) for x in os.environ["K_DVE_SQ"].split(",") if x != ""}
    if os.environ.get("K_WARM") is not None:
        warm = int(os.environ["K_WARM"])
    return gp, pe, dq, warm


_program = None


def _build_program():
    try:
        import concourse.bass as bass  # noqa: F401
    except ImportError:
        sys.path.insert(0, "/opt/trn_rl_repo")
        import concourse.bass as bass  # noqa: F401
    from concourse import bacc, mybir, tile
    from concourse.tile import add_dep_helper

    f32 = mybir.dt.float32
    bf16 = mybir.dt.bfloat16
    f8 = mybir.dt.float8e4
    AF = mybir.ActivationFunctionType
    OP = mybir.AluOpType
    AX = mybir.AxisListType

    gp_sub, pe_sub, dve_sq, warm_mm = _routing()
    assert 16 not in pe_sub, "tail chunk cannot go through the PE-sub path"

    nc = bacc.Bacc("TRN2", target_bir_lowering=False, debug=False,
                   num_devices=N_CORES)

    data8 = nc.dram_tensor("data8", (P, F8), f8, kind="ExternalInput")
    aux16 = nc.dram_tensor("aux16", (P, F16), bf16, kind="ExternalInput")
    out = nc.dram_tensor("out", (1, 1), f32, kind="ExternalOutput")

    with tile.TileContext(nc) as tc:
        with tc.tile_pool(name="aux", bufs=1) as apool, \
             tc.tile_pool(name="attr", bufs=1) as tpool, \
             tc.tile_pool(name="stream", bufs=1) as spool, \
             tc.tile_pool(name="diff", bufs=4) as dpool, \
             tc.tile_pool(name="sq", bufs=4) as qpool, \
             tc.tile_pool(name="wrk", bufs=1) as wpool, \
             tc.tile_pool(name="pscol", bufs=1, space="PSUM") as pcol, \
             tc.tile_pool(name="psw", bufs=1, space="PSUM") as psw, \
             tc.tile_pool(name="psd", bufs=2, space="PSUM") as psd, \
             tc.tile_pool(name="psf", bufs=1, space="PSUM") as psf:

            dve_ops = []
            act_ops = []
            pe_ops = []
            gp_ops = []

            # ---------------- PE warmup (no DMA dependency)
            wtile = wpool.tile([P, 512], bf16)
            gp_ops.append(nc.gpsimd.memset(wtile[:], 1.0))
            ones32 = wpool.tile([32, 1], f32)
            gp_ops.append(nc.gpsimd.memset(ones32[:], 1.0))
            if warm_mm:
                wps_warm = psw.tile([32, 512], f32, tag="wwarm")
                for i in range(warm_mm):
                    pe_ops.append(nc.tensor.matmul(
                        wps_warm[:], wtile[:, 0:32], wtile[:],
                        start=True, stop=True))

            # ---------------- DMA: aux, attr+asub, then the 17 chunk pairs
            aux = apool.tile([P, F16], bf16)
            attr = tpool.tile([P, ATTR_LEN + 64], f8)
            dmas = [nc.sync.dma_start(aux[:], aux16.ap())]
            dmas.append(nc.sync.dma_start(attr[:],
                                          data8.ap()[:, 0:ATTR_LEN + 64]))
            chtiles = []
            c0 = 0
            for gi, gsize in enumerate(DMA_GROUPS):
                t = spool.tile([P, gsize * 2 * CHW], f8, tag=f"g{gi}")
                for k in range(gsize):
                    chtiles.append(t[:, k * 2 * CHW:(k + 1) * 2 * CHW])
                off = CH_OFF + c0 * 2 * CHW
                dmas.append(nc.sync.dma_start(
                    t[:], data8.ap()[:, off:off + gsize * 2 * CHW]))
                c0 += gsize
            assert c0 == NCH + 1
            for i in range(len(dmas) - 1):
                add_dep_helper(dmas[i + 1].ins, dmas[i].ins, sync=False,
                               reason="DMA issue order")

            ea_sb = aux[:, EA_OFF:EA_OFF + 384]
            anum_sb = aux[:, ANUM_OFF:ANUM_OFF + 6]
            awin = aux[:, AWIN_OFF:AWIN_OFF + 63]
            zt = aux[:, ZT_OFF:ZT_OFF + 48]
            asub = attr[:, ATTR_LEN:ATTR_LEN + 64]

            # ---------------- weight chain pieces (interleaved into DVE)
            a_sb = wpool.tile([P, 6], f32)
            tot = wpool.tile([P, 1], f32)
            rec = wpool.tile([P, 6], f32)
            ivb = wpool.tile([P, 6], f32)
            aw0 = wpool.tile([P, 128], bf16)
            aw1 = wpool.tile([P, 128], bf16)
            sinh = wpool.tile([P, 384], bf16)
            ssq = wpool.tile([P, 384], bf16)
            ang = wpool.tile([P, 128], bf16)
            w_nat = wpool.tile([P, 128], bf16)

            def w_iv():
                dve_ops.append(nc.vector.tensor_copy(a_sb[:], anum_sb))
                dve_ops.append(nc.vector.tensor_reduce(
                    tot[:], a_sb[:], axis=AX.X, op=OP.add))
                dve_ops.append(nc.vector.reciprocal(rec[:], a_sb[:]))
                dve_ops.append(nc.vector.tensor_scalar_mul(
                    ivb[:], rec[:], tot[:, 0:1]))

            def w_attr():
                dve_ops.append(nc.vector.tensor_scalar_mul(
                    aw0[:], attr[:, 0:128], ivb[:, 0:1]))
                cur, nxt = aw0, aw1
                for j in range(1, 6):
                    dve_ops.append(nc.vector.scalar_tensor_tensor(
                        nxt[:], attr[:, j * 128:(j + 1) * 128],
                        ivb[:, j:j + 1], cur[:], op0=OP.mult, op1=OP.add))
                    cur, nxt = nxt, cur
                return cur

            def w_angle(awt):
                dve_ops.append(nc.vector.tensor_add(
                    ang[:], ssq[:, 0:128], ssq[:, 128:256]))
                dve_ops.append(nc.vector.scalar_tensor_tensor(
                    ang[:], ssq[:, 256:384], 1.0, ang[:],
                    op0=OP.bypass, op1=OP.add))
                dve_ops.append(nc.vector.scalar_tensor_tensor(
                    w_nat[:], ang[:], 2.0, awt[:], op0=OP.mult, op1=OP.mult))

            act_ops.append(nc.scalar.activation(sinh[:], ea_sb, AF.Sin,
                                                bias=0.0, scale=0.5))
            act_ops.append(nc.scalar.activation(ssq[:], sinh[:], AF.Square))

            # ---------------- main loop
            colsq = pcol.tile([32, 512], f32)
            wps = psw.tile([32, 512], f32, tag="wsel")
            w_sb = wpool.tile([32, 512], f32)
            n_mm = 2 * (NCH + 1)
            mm_idx = 0
            awt = [None]
            w_copy_pending = [True]

            def emit_selectors():
                for beta in range(4):
                    sel = aux[:, ASEL_OFF + 32 * beta:
                              ASEL_OFF + 32 * beta + 32]
                    pe_ops.append(nc.tensor.matmul(
                        wps[:, 128 * beta:128 * beta + 128], sel, w_nat[:],
                        start=True, stop=True))

            for c in range(NCH + 1):
                t = chtiles[c]
                if c in pe_sub:
                    # PE subtract: pair-interleaved halves -> psum diff
                    for h in range(2):
                        pdiff = psd.tile([P, 512], f32, tag="pd")
                        for half in range(2):
                            mvo = (2 * h + half) * 512
                            pe_ops.append(nc.tensor.matmul(
                                pdiff[64 * half:64 * half + 64, :],
                                asub, t[:, mvo:mvo + 512],
                                start=True, stop=True))
                        sq = qpool.tile([P, 512], bf16, tag="sqh")
                        act_ops.append(nc.scalar.activation(
                            sq[:], pdiff[:], AF.Square))
                        r = 2 * c + h
                        pe_ops.append(nc.tensor.matmul(
                            colsq[:], awin[:, 31 - r:63 - r], sq[:],
                            start=(mm_idx == 0), stop=(mm_idx == n_mm - 1)))
                        mm_idx += 1
                else:
                    df = dpool.tile([P, CHW], bf16, tag="diff")
                    if c in gp_sub:
                        gp_ops.append(nc.gpsimd.tensor_sub(
                            df[:], t[:, 0:CHW], t[:, CHW:2 * CHW]))
                    else:
                        dve_ops.append(nc.vector.tensor_sub(
                            df[:], t[:, 0:CHW], t[:, CHW:2 * CHW]))
                    sq = qpool.tile([P, CHW], bf16, tag="sq")
                    if c in dve_sq:
                        dve_ops.append(nc.vector.tensor_mul(
                            sq[:], df[:], df[:]))
                    else:
                        act_ops.append(nc.scalar.activation(
                            sq[:], df[:], AF.Square))
                    for h in range(2):
                        mv = sq[:, h * 512:(h + 1) * 512]
                        if c < NCH:
                            r = 2 * c + h
                            lhsT = awin[:, 31 - r:63 - r]
                        else:
                            g = h
                            lhsT = zt[:, 16 - 16 * g:48 - 16 * g]
                        pe_ops.append(nc.tensor.matmul(
                            colsq[:], lhsT, mv,
                            start=(mm_idx == 0), stop=(mm_idx == n_mm - 1)))
                        mm_idx += 1
                # weave the weight chain between early chunks
                if c == 0:
                    w_iv()
                elif c == 1:
                    awt[0] = w_attr()
                elif c == 2:
                    w_angle(awt[0])
                elif c == 4:
                    emit_selectors()
                elif c == 5:
                    act_ops.append(nc.scalar.activation(
                        w_sb[:], wps[:], AF.Copy))
                    w_copy_pending[0] = False
            assert not w_copy_pending[0]

            # ---------------- final weighted reduce
            scr = wpool.tile([32, 512], f32)
            part = wpool.tile([32, 1], f32)
            dve_ops.append(nc.vector.scalar_tensor_tensor(
                scr[:], colsq[:], 1.0, w_sb[:], op0=OP.bypass, op1=OP.mult,
                accum_out=part[:]))
            fin = psf.tile([1, 1], f32)
            pe_ops.append(nc.tensor.matmul(fin[:], ones32[:], part[:],
                                           start=True, stop=True))
            res = wpool.tile([1, 1], f32)
            dve_ops.append(nc.vector.tensor_copy(res[:], fin[:]))
            nc.sync.dma_start(out.ap(), res[:])

            # ---------------- per-engine issue-order pins
            for ops in (dve_ops, act_ops, pe_ops, gp_ops):
                for i in range(len(ops) - 1):
                    add_dep_helper(ops[i + 1].ins, ops[i].ins, sync=False,
                                   reason="engine order")

    nc.compile()
    return nc


def _get_program():
    global _program
    if _program is None:
        _program = _build_program()
    return _program


def _make_in_maps(inp, label, ea, attribute, attribute_num):
    import ml_dtypes
    e4 = ml_dtypes.float8_e4m3
    bf = ml_dtypes.bfloat16
    _, pe_sub, _, _ = _routing()
    inp = np.asarray(inp, dtype=np.float32)
    label = np.asarray(label, dtype=np.float32)
    ea = np.asarray(ea, dtype=np.float32)
    attribute = np.asarray(attribute, dtype=np.int32)
    anum_row = np.asarray(attribute_num, dtype=np.float32).reshape(6)

    # constant fields (same for all cores)
    awin = np.zeros((P, 63), dtype=np.float32)
    awin[:, 31] = 1.0
    ztm = np.zeros((P, 48), dtype=np.float32)
    for k in range(16):
        ztm[8 * k:8 * k + 8, 16 + k] = 1.0
    asel = np.zeros((P, 128), dtype=np.float32)
    for beta in range(4):
        for r in range(32):
            asel[4 * r + beta, 32 * beta + r] = 1.0
    asub = np.zeros((P, 64), dtype=np.float32)
    for k in range(64):
        asub[2 * k, k] = 1.0
        asub[2 * k + 1, k] = -1.0

    in_maps = []
    for c in range(N_CORES):
        sl = slice(c * BS, (c + 1) * BS)
        xi = inp[:, sl]
        xl = label[:, sl]
        d8 = np.empty((P, F8), dtype=e4)
        d8[:, ATTR_OFF:ATTR_OFF + ATTR_LEN] = (
            attribute[:, sl].reshape(6, P, 128).transpose(1, 0, 2)
            .reshape(P, ATTR_LEN).astype(e4))
        d8[:, ASUB_OFF:ASUB_OFF + 64] = asub.astype(e4)
        mi = xi[0:128].astype(e4)
        ml = xl[0:128].astype(e4)
        for k in range(NCH):
            off = CH_OFF + k * 2 * CHW
            if k in pe_sub:
                # pair-interleave: halves of 512 cols, partition 2j = inp
                # d-row j(+64), 2j+1 = label d-row j(+64)
                blk = np.empty((P, 2 * CHW), dtype=e4)
                for h in range(2):
                    for half in range(2):
                        src_i = mi[64 * half:64 * half + 64,
                                   k * CHW + h * 512:k * CHW + (h + 1) * 512]
                        src_l = ml[64 * half:64 * half + 64,
                                   k * CHW + h * 512:k * CHW + (h + 1) * 512]
                        pi = np.empty((128, 512), dtype=e4)
                        pi[0::2] = src_i
                        pi[1::2] = src_l
                        blk[:, (2 * h + half) * 512:
                            (2 * h + half + 1) * 512] = pi
                d8[:, off:off + 2 * CHW] = blk
            else:
                d8[:, off:off + CHW] = mi[:, k * CHW:(k + 1) * CHW]
                d8[:, off + CHW:off + 2 * CHW] = ml[:, k * CHW:(k + 1) * CHW]
        toff = CH_OFF + NCH * 2 * CHW
        ti = (xi[128:136].reshape(8, 2, 16, 512).transpose(2, 0, 1, 3)
              .reshape(P, NTAIL).astype(e4))
        tl = (xl[128:136].reshape(8, 2, 16, 512).transpose(2, 0, 1, 3)
              .reshape(P, NTAIL).astype(e4))
        d8[:, toff:toff + NTAIL] = ti
        d8[:, toff + NTAIL:toff + 2 * NTAIL] = tl

        a16 = np.zeros((P, F16), dtype=bf)
        a16[:, EA_OFF:EA_OFF + 384] = (
            ea[:, sl].reshape(3, P, 128).transpose(1, 0, 2)
            .reshape(P, 384).astype(bf))
        a16[:, ANUM_OFF:ANUM_OFF + 6] = anum_row.astype(bf)
        a16[:, AWIN_OFF:AWIN_OFF + 63] = awin.astype(bf)
        a16[:, ZT_OFF:ZT_OFF + 48] = ztm.astype(bf)
        a16[:, ASEL_OFF:ASEL_OFF + 128] = asel.astype(bf)
        in_maps.append({"data8": d8, "aux16": a16})
    return in_maps


def run(inputs, trace=False, trace_cores=None):
    """Run on hardware; returns (result_scalar, BassKernelResults)."""
    try:
        from concourse.bass_utils import run_bass_kernel_spmd
    except ImportError:
        sys.path.insert(0, "/opt/trn_rl_repo")
        from concourse.bass_utils import run_bass_kernel_spmd
    nc = _get_program()
    in_maps = _make_in_maps(**inputs)
    kwargs = {}
    if trace:
        kwargs["trace"] = True
        if trace_cores is not None:
            kwargs["trace_cores"] = trace_cores
    res = run_bass_kernel_spmd(nc, in_maps, core_ids=list(range(N_CORES)),
                               **kwargs)
    total = 0.0
    for r in res.results:
        total += float(r["out"].astype(np.float64).sum())
    value = np.asarray(total / (D * B), dtype=np.float32)
    return value, res


def kernel(**inputs):
    value, _ = run(inputs)
    return value


# revision 9
# speedup vs baseline: 1.2411x; 1.2411x over previous
"""Weighted-MSE loss kernel (nn_LossWithEuler) for 8 Trainium2 NeuronCores.

loss = mean(weight[b] * (inp[d,b] - label[d,b])^2)
  weight[b]  = attr_w[b] * angle_w[b]
  attr_w[b]  = sum_j (attribute[j,b]==1) * (sum(attribute_num)/attribute_num[j])
  angle_w[b] = sum_j (1 - cos(ea[j,b])) = sum_j 2*sin(ea[j,b]/2)^2

Sharding: batch axis B=131072 split across 8 cores (16384 b's each).

inp/label are host-cast to fp8-e4m3 (quantization error on the final mean
~1.6e-3 vs the 2e-2 gate; halves the HBM stream vs bf16) and delivered in
COLUMN layout: partition = d (rows 0..127), free = b.  The leftover 8
d-rows (128..135) are repacked into 1024 "tail" columns of 16 b-blocks
(8 d's each).  In this layout the d-reduction is a ones-matmul on the
otherwise-idle TensorEngine: 32 sliding-window A-matmuls (512 cols each)
+ 2 block-A tail matmuls accumulate sum_d diff^2 per b into ONE [32,512]
f32 PSUM tile (colsq[r,q] = b = r*512+q).  A PE warmup spin at kernel
start keeps the PE at its 2.4 GHz HAM clock.  Chunks routed to PE_SUB are
delivered pair-interleaved (partition 2k = inp d-row, 2k+1 = label d-row)
and subtracted BY the PE with a fixed +-1 stationary; ACT squares those
from PSUM.  Remaining chunks: DVE subtracts (fp8 TT runs 1x), ACT (+DVE
for the last chunks) squares.  GPSIMD is kept idle: concurrent GPSIMD
tensor ops slow DVE ops ~2.5x (SBUF contention, measured).  The per-b
weight is computed in the natural [128,128] layout (b = p*128+f) and
shuffled to the colsq layout by 4 PE selector matmuls (w'[r,128*beta+f] =
w_nat[4r+beta, f]).  Final: one fused STT (colsq * w' with free-dim
accumulate), a [32]->[1] ones matmul, one f32 DMA out per core; host sums
8 partials / (D*B).
"""

import os
import sys
import numpy as np

D = 136
B = 131072
N_CORES = 8
BS = B // N_CORES       # 16384 b's per core
P = 128
NTAIL = BS // 16        # 1024 tail columns (16 b's x 8 d's each)
NCH = 16                # main chunks of 1024 cols
CHW = 1024              # chunk width (2 PE windows of 512)

# data8 field offsets (fp8 bytes per partition)
ATTR_OFF = 0
ATTR_LEN = 6 * 128      # 768
ASUB_OFF = ATTR_OFF + ATTR_LEN          # [128, 64] +-1 pair-subtract matrix
CH_OFF = ASUB_OFF + 64                  # chunks: 17 x (i 1024 | l 1024)
F8 = CH_OFF + (NCH + 1) * 2 * CHW

# aux16 field offsets (bf16 elems per partition)
EA_OFF = 0              # [128, 384]
ANUM_OFF = 384          # [128, 6]
AWIN_OFF = 392          # [128, 63]  main sliding ones-window
ZT_OFF = 456            # [128, 48]  tail block window
ASEL_OFF = 504          # [128, 4*32] selector matrices
F16 = 640

# engine routing (chunk indices 0..15 main, 16 = tail)
GP_SUB = set()                  # gpsimd sub chunks (hurts DVE; keep empty)
PE_SUB = {1, 3, 5, 7}           # pair-interleaved chunks subtracted on PE
DVE_SQ = {12, 13, 14, 15, 16}   # square chunks on DVE (rest on ACT)
WARM_MM = 5                     # PE warmup matmuls (512 cols each)
DMA_GROUPS = [1, 1, 2, 2, 2, 3, 3, 3]  # chunk DMAs batched to cut issue cost
PE_FILL = 2                     # dummy matmuls after each chunk's windows (keeps HAM hot)


def _routing():
    gp = GP_SUB
    pe = PE_SUB
    dq = DVE_SQ
    warm = WARM_MM
    if os.environ.get("K_GP_SUB") is not None:
        gp = {int(x) for x in os.environ["K_GP_SUB"].split(",") if x != ""}
    if os.environ.get("K_PE_SUB") is not None:
        pe = {int(x) for x in os.environ["K_PE_SUB"].split(",") if x != ""}
    if os.environ.get("K_DVE_SQ") is not None:
        dq = {int(x) for x in os.environ["K_DVE_SQ"].split(",") if x != ""}
    if os.environ.get("K_WARM") is not None:
        warm = int(os.environ["K_WARM"])
    fill = PE_FILL
    if os.environ.get("K_FILL") is not None:
        fill = int(os.environ["K_FILL"])
    return gp, pe, dq, warm, fill


_program = None


def _build_program():
    try:
        import concourse.bass as bass  # noqa: F401
    except ImportError:
        sys.path.insert(0, "/opt/trn_rl_repo")
        import concourse.bass as bass  # noqa: F401
    from concourse import bacc, mybir, tile
    from concourse.tile import add_dep_helper

    f32 = mybir.dt.float32
    bf16 = mybir.dt.bfloat16
    f8 = mybir.dt.float8e4
    AF = mybir.ActivationFunctionType
    OP = mybir.AluOpType
    AX = mybir.AxisListType

    gp_sub, pe_sub, dve_sq, warm_mm, pe_fill = _routing()
    assert 16 not in pe_sub, "tail chunk cannot go through the PE-sub path"

    nc = bacc.Bacc("TRN2", target_bir_lowering=False, debug=False,
                   num_devices=N_CORES)

    data8 = nc.dram_tensor("data8", (P, F8), f8, kind="ExternalInput")
    aux16 = nc.dram_tensor("aux16", (P, F16), bf16, kind="ExternalInput")
    out = nc.dram_tensor("out", (1, 1), f32, kind="ExternalOutput")

    with tile.TileContext(nc) as tc:
        with tc.tile_pool(name="aux", bufs=1) as apool, \
             tc.tile_pool(name="attr", bufs=1) as tpool, \
             tc.tile_pool(name="stream", bufs=1) as spool, \
             tc.tile_pool(name="diff", bufs=6) as dpool, \
             tc.tile_pool(name="sq", bufs=6) as qpool, \
             tc.tile_pool(name="wrk", bufs=1) as wpool, \
             tc.tile_pool(name="pscol", bufs=1, space="PSUM") as pcol, \
             tc.tile_pool(name="psw", bufs=1, space="PSUM") as psw, \
             tc.tile_pool(name="psd", bufs=2, space="PSUM") as psd, \
             tc.tile_pool(name="psf", bufs=1, space="PSUM") as psf:

            dve_ops = []
            act_ops = []
            pe_ops = []
            gp_ops = []

            # ---------------- PE warmup (no DMA dependency)
            wtile = wpool.tile([P, 512], bf16)
            gp_ops.append(nc.gpsimd.memset(wtile[:], 1.0))
            ones32 = wpool.tile([32, 1], f32)
            gp_ops.append(nc.gpsimd.memset(ones32[:], 1.0))
            wps_warm = psw.tile([32, 512], f32, tag="wwarm")
            if warm_mm:
                for i in range(warm_mm):
                    pe_ops.append(nc.tensor.matmul(
                        wps_warm[:], wtile[:, 0:32], wtile[:],
                        start=True, stop=True))

            # ---------------- DMA: aux, attr+asub, then the 17 chunk pairs
            aux = apool.tile([P, F16], bf16)
            attr = tpool.tile([P, ATTR_LEN + 64], f8)
            dmas = [nc.sync.dma_start(aux[:], aux16.ap())]
            dmas.append(nc.sync.dma_start(attr[:],
                                          data8.ap()[:, 0:ATTR_LEN + 64]))
            chtiles = []
            c0 = 0
            for gi, gsize in enumerate(DMA_GROUPS):
                t = spool.tile([P, gsize * 2 * CHW], f8, tag=f"g{gi}")
                for k in range(gsize):
                    chtiles.append(t[:, k * 2 * CHW:(k + 1) * 2 * CHW])
                off = CH_OFF + c0 * 2 * CHW
                dmas.append(nc.sync.dma_start(
                    t[:], data8.ap()[:, off:off + gsize * 2 * CHW]))
                c0 += gsize
            assert c0 == NCH + 1
            for i in range(len(dmas) - 1):
                add_dep_helper(dmas[i + 1].ins, dmas[i].ins, sync=False,
                               reason="DMA issue order")

            ea_sb = aux[:, EA_OFF:EA_OFF + 384]
            anum_sb = aux[:, ANUM_OFF:ANUM_OFF + 6]
            awin = aux[:, AWIN_OFF:AWIN_OFF + 63]
            zt = aux[:, ZT_OFF:ZT_OFF + 48]
            asub = attr[:, ATTR_LEN:ATTR_LEN + 64]

            # ---------------- weight chain pieces (interleaved into DVE)
            a_sb = wpool.tile([P, 6], f32)
            tot = wpool.tile([P, 1], f32)
            rec = wpool.tile([P, 6], f32)
            ivb = wpool.tile([P, 6], f32)
            aw0 = wpool.tile([P, 128], bf16)
            aw1 = wpool.tile([P, 128], bf16)
            sinh = wpool.tile([P, 384], bf16)
            ssq = wpool.tile([P, 384], bf16)
            ang = wpool.tile([P, 128], bf16)
            w_nat = wpool.tile([P, 128], bf16)

            def w_iv():
                dve_ops.append(nc.vector.tensor_copy(a_sb[:], anum_sb))
                dve_ops.append(nc.vector.tensor_reduce(
                    tot[:], a_sb[:], axis=AX.X, op=OP.add))
                dve_ops.append(nc.vector.reciprocal(rec[:], a_sb[:]))
                dve_ops.append(nc.vector.tensor_scalar_mul(
                    ivb[:], rec[:], tot[:, 0:1]))

            def w_attr():
                dve_ops.append(nc.vector.tensor_scalar_mul(
                    aw0[:], attr[:, 0:128], ivb[:, 0:1]))
                cur, nxt = aw0, aw1
                for j in range(1, 6):
                    dve_ops.append(nc.vector.scalar_tensor_tensor(
                        nxt[:], attr[:, j * 128:(j + 1) * 128],
                        ivb[:, j:j + 1], cur[:], op0=OP.mult, op1=OP.add))
                    cur, nxt = nxt, cur
                return cur

            def w_angle(awt):
                dve_ops.append(nc.vector.tensor_add(
                    ang[:], ssq[:, 0:128], ssq[:, 128:256]))
                dve_ops.append(nc.vector.scalar_tensor_tensor(
                    ang[:], ssq[:, 256:384], 1.0, ang[:],
                    op0=OP.bypass, op1=OP.add))
                dve_ops.append(nc.vector.scalar_tensor_tensor(
                    w_nat[:], ang[:], 2.0, awt[:], op0=OP.mult, op1=OP.mult))

            act_ops.append(nc.scalar.activation(sinh[:], ea_sb, AF.Sin,
                                                bias=0.0, scale=0.5))
            act_ops.append(nc.scalar.activation(ssq[:], sinh[:], AF.Square))

            # ---------------- main loop
            colsq = pcol.tile([32, 512], f32)
            wps = psw.tile([32, 512], f32, tag="wsel")
            w_sb = wpool.tile([32, 512], f32)
            n_mm = 2 * (NCH + 1)
            mm_idx = 0
            awt = [None]
            w_copy_pending = [True]

            def emit_selectors():
                for beta in range(4):
                    sel = aux[:, ASEL_OFF + 32 * beta:
                              ASEL_OFF + 32 * beta + 32]
                    pe_ops.append(nc.tensor.matmul(
                        wps[:, 128 * beta:128 * beta + 128], sel, w_nat[:],
                        start=True, stop=True))

            for c in range(NCH + 1):
                t = chtiles[c]
                if c in pe_sub:
                    # PE subtract: pair-interleaved halves -> psum diff
                    for h in range(2):
                        pdiff = psd.tile([P, 512], f32, tag="pd")
                        for half in range(2):
                            mvo = (2 * h + half) * 512
                            pe_ops.append(nc.tensor.matmul(
                                pdiff[64 * half:64 * half + 64, :],
                                asub, t[:, mvo:mvo + 512],
                                start=True, stop=True))
                        sq = qpool.tile([P, 512], bf16, tag="sqh")
                        act_ops.append(nc.scalar.activation(
                            sq[:], pdiff[:], AF.Square))
                        r = 2 * c + h
                        pe_ops.append(nc.tensor.matmul(
                            colsq[:], awin[:, 31 - r:63 - r], sq[:],
                            start=(mm_idx == 0), stop=(mm_idx == n_mm - 1)))
                        mm_idx += 1
                else:
                    df = dpool.tile([P, CHW], bf16, tag="diff")
                    if c in gp_sub:
                        gp_ops.append(nc.gpsimd.tensor_sub(
                            df[:], t[:, 0:CHW], t[:, CHW:2 * CHW]))
                    else:
                        dve_ops.append(nc.vector.tensor_sub(
                            df[:], t[:, 0:CHW], t[:, CHW:2 * CHW]))
                    sq = qpool.tile([P, CHW], bf16, tag="sq")
                    if c in dve_sq:
                        dve_ops.append(nc.vector.tensor_mul(
                            sq[:], df[:], df[:]))
                    else:
                        act_ops.append(nc.scalar.activation(
                            sq[:], df[:], AF.Square))
                    for h in range(2):
                        mv = sq[:, h * 512:(h + 1) * 512]
                        if c < NCH:
                            r = 2 * c + h
                            lhsT = awin[:, 31 - r:63 - r]
                        else:
                            g = h
                            lhsT = zt[:, 16 - 16 * g:48 - 16 * g]
                        pe_ops.append(nc.tensor.matmul(
                            colsq[:], lhsT, mv,
                            start=(mm_idx == 0), stop=(mm_idx == n_mm - 1)))
                        mm_idx += 1
                for _ in range(pe_fill):
                    pe_ops.append(nc.tensor.matmul(
                        wps_warm[:], wtile[:, 0:32], wtile[:],
                        start=True, stop=True))
                # weave the weight chain between early chunks
                if c == 0:
                    w_iv()
                elif c == 1:
                    awt[0] = w_attr()
                elif c == 2:
                    w_angle(awt[0])
                elif c == 4:
                    emit_selectors()
                elif c == 5:
                    act_ops.append(nc.scalar.activation(
                        w_sb[:], wps[:], AF.Copy))
                    w_copy_pending[0] = False
            assert not w_copy_pending[0]

            # ---------------- final weighted reduce
            scr = wpool.tile([32, 512], f32)
            part = wpool.tile([32, 1], f32)
            dve_ops.append(nc.vector.scalar_tensor_tensor(
                scr[:], colsq[:], 1.0, w_sb[:], op0=OP.bypass, op1=OP.mult,
                accum_out=part[:]))
            fin = psf.tile([1, 1], f32)
            pe_ops.append(nc.tensor.matmul(fin[:], ones32[:], part[:],
                                           start=True, stop=True))
            res = wpool.tile([1, 1], f32)
            dve_ops.append(nc.vector.tensor_copy(res[:], fin[:]))
            nc.sync.dma_start(out.ap(), res[:])

            # ---------------- per-engine issue-order pins
            for ops in (dve_ops, act_ops, pe_ops, gp_ops):
                for i in range(len(ops) - 1):
                    add_dep_helper(ops[i + 1].ins, ops[i].ins, sync=False,
                                   reason="engine order")

    nc.compile()
    return nc


def _get_program():
    global _program
    if _program is None:
        _program = _build_program()
    return _program


def _make_in_maps(inp, label, ea, attribute, attribute_num):
    import ml_dtypes
    e4 = ml_dtypes.float8_e4m3
    bf = ml_dtypes.bfloat16
    _, pe_sub, _, _, _ = _routing()
    inp = np.asarray(inp, dtype=np.float32)
    label = np.asarray(label, dtype=np.float32)
    ea = np.asarray(ea, dtype=np.float32)
    attribute = np.asarray(attribute, dtype=np.int32)
    anum_row = np.asarray(attribute_num, dtype=np.float32).reshape(6)

    # constant fields (same for all cores)
    awin = np.zeros((P, 63), dtype=np.float32)
    awin[:, 31] = 1.0
    ztm = np.zeros((P, 48), dtype=np.float32)
    for k in range(16):
        ztm[8 * k:8 * k + 8, 16 + k] = 1.0
    asel = np.zeros((P, 128), dtype=np.float32)
    for beta in range(4):
        for r in range(32):
            asel[4 * r + beta, 32 * beta + r] = 1.0
    asub = np.zeros((P, 64), dtype=np.float32)
    for k in range(64):
        asub[2 * k, k] = 1.0
        asub[2 * k + 1, k] = -1.0

    in_maps = []
    for c in range(N_CORES):
        sl = slice(c * BS, (c + 1) * BS)
        xi = inp[:, sl]
        xl = label[:, sl]
        d8 = np.empty((P, F8), dtype=e4)
        d8[:, ATTR_OFF:ATTR_OFF + ATTR_LEN] = (
            attribute[:, sl].reshape(6, P, 128).transpose(1, 0, 2)
            .reshape(P, ATTR_LEN).astype(e4))
        d8[:, ASUB_OFF:ASUB_OFF + 64] = asub.astype(e4)
        mi = xi[0:128].astype(e4)
        ml = xl[0:128].astype(e4)
        for k in range(NCH):
            off = CH_OFF + k * 2 * CHW
            if k in pe_sub:
                # pair-interleave: halves of 512 cols, partition 2j = inp
                # d-row j(+64), 2j+1 = label d-row j(+64)
                blk = np.empty((P, 2 * CHW), dtype=e4)
                for h in range(2):
                    for half in range(2):
                        src_i = mi[64 * half:64 * half + 64,
                                   k * CHW + h * 512:k * CHW + (h + 1) * 512]
                        src_l = ml[64 * half:64 * half + 64,
                                   k * CHW + h * 512:k * CHW + (h + 1) * 512]
                        pi = np.empty((128, 512), dtype=e4)
                        pi[0::2] = src_i
                        pi[1::2] = src_l
                        blk[:, (2 * h + half) * 512:
                            (2 * h + half + 1) * 512] = pi
                d8[:, off:off + 2 * CHW] = blk
            else:
                d8[:, off:off + CHW] = mi[:, k * CHW:(k + 1) * CHW]
                d8[:, off + CHW:off + 2 * CHW] = ml[:, k * CHW:(k + 1) * CHW]
        toff = CH_OFF + NCH * 2 * CHW
        ti = (xi[128:136].reshape(8, 2, 16, 512).transpose(2, 0, 1, 3)
              .reshape(P, NTAIL).astype(e4))
        tl = (xl[128:136].reshape(8, 2, 16, 512).transpose(2, 0, 1, 3)
              .reshape(P, NTAIL).astype(e4))
        d8[:, toff:toff + NTAIL] = ti
        d8[:, toff + NTAIL:toff + 2 * NTAIL] = tl

        a16 = np.zeros((P, F16), dtype=bf)
        a16[:, EA_OFF:EA_OFF + 384] = (
            ea[:, sl].reshape(3, P, 128).transpose(1, 0, 2)
            .reshape(P, 384).astype(bf))
        a16[:, ANUM_OFF:ANUM_OFF + 6] = anum_row.astype(bf)
        a16[:, AWIN_OFF:AWIN_OFF + 63] = awin.astype(bf)
        a16[:, ZT_OFF:ZT_OFF + 48] = ztm.astype(bf)
        a16[:, ASEL_OFF:ASEL_OFF + 128] = asel.astype(bf)
        in_maps.append({"data8": d8, "aux16": a16})
    return in_maps


def run(inputs, trace=False, trace_cores=None):
    """Run on hardware; returns (result_scalar, BassKernelResults)."""
    try:
        from concourse.bass_utils import run_bass_kernel_spmd
    except ImportError:
        sys.path.insert(0, "/opt/trn_rl_repo")
        from concourse.bass_utils import run_bass_kernel_spmd
    nc = _get_program()
    in_maps = _make_in_maps(**inputs)
    kwargs = {}
    if trace:
        kwargs["trace"] = True
        if trace_cores is not None:
            kwargs["trace_cores"] = trace_cores
    res = run_bass_kernel_spmd(nc, in_maps, core_ids=list(range(N_CORES)),
                               **kwargs)
    total = 0.0
    for r in res.results:
        total += float(r["out"].astype(np.float64).sum())
    value = np.asarray(total / (D * B), dtype=np.float32)
    return value, res


def kernel(**inputs):
    value, _ = run(inputs)
    return value


# revision 10
# speedup vs baseline: 1.4094x; 1.1356x over previous
"""Weighted-MSE loss kernel (nn_LossWithEuler) for 8 Trainium2 NeuronCores.

loss = mean(weight[b] * (inp[d,b] - label[d,b])^2)
  weight[b]  = attr_w[b] * angle_w[b]
  attr_w[b]  = sum_j (attribute[j,b]==1) * (sum(attribute_num)/attribute_num[j])
  angle_w[b] = sum_j (1 - cos(ea[j,b])) = sum_j 2*sin(ea[j,b]/2)^2

Sharding: batch axis B=131072 split across 8 cores (16384 b's each).

inp/label are host-cast to fp8-e4m3 (quantization error on the final mean
~1.6e-3 vs the 2e-2 gate; halves the HBM stream vs bf16) and delivered in
COLUMN layout: partition = d (rows 0..127), free = b.  The leftover 8
d-rows (128..135) are repacked into 1024 "tail" columns of 16 b-blocks
(8 d's each).  In this layout the d-reduction is a ones-matmul on the
otherwise-idle TensorEngine: 32 sliding-window A-matmuls (512 cols each)
+ 2 block-A tail matmuls accumulate sum_d diff^2 per b into ONE [32,512]
f32 PSUM tile (colsq[r,q] = b = r*512+q).  A PE warmup spin at kernel
start keeps the PE at its 2.4 GHz HAM clock.  Chunks routed to PE_SUB are
delivered pair-interleaved (partition 2k = inp d-row, 2k+1 = label d-row)
and subtracted BY the PE with a fixed +-1 stationary; ACT squares those
from PSUM.  Remaining chunks: DVE subtracts (fp8 TT runs 1x), ACT (+DVE
for the last chunks) squares.  GPSIMD is kept idle: concurrent GPSIMD
tensor ops slow DVE ops ~2.5x (SBUF contention, measured).  The per-b
weight is computed in the natural [128,128] layout (b = p*128+f) and
shuffled to the colsq layout by 4 PE selector matmuls (w'[r,128*beta+f] =
w_nat[4r+beta, f]).  Final: one fused STT (colsq * w' with free-dim
accumulate), a [32]->[1] ones matmul, one f32 DMA out per core; host sums
8 partials / (D*B).
"""

import os
import sys
import numpy as np

D = 136
B = 131072
N_CORES = 8
BS = B // N_CORES       # 16384 b's per core
P = 128
NTAIL = BS // 16        # 1024 tail columns (16 b's x 8 d's each)
NCH = 16                # main chunks of 1024 cols
CHW = 1024              # chunk width (2 PE windows of 512)

# data8 field offsets (fp8 bytes per partition)
ATTR_OFF = 0
ATTR_LEN = 6 * 128      # 768
ASUB_OFF = ATTR_OFF + ATTR_LEN          # [128, 64] +-1 pair-subtract matrix
CH_OFF = ASUB_OFF + 64                  # chunks: 17 x (i 1024 | l 1024)
F8 = CH_OFF + (NCH + 1) * 2 * CHW

# aux16 field offsets (bf16 elems per partition)
EA_OFF = 0              # [128, 384]
ANUM_OFF = 384          # [128, 6]
AWIN_OFF = 392          # [128, 63]  main sliding ones-window
ZT_OFF = 456            # [128, 48]  tail block window
ASEL_OFF = 504          # [128, 4*32] selector matrices
F16 = 640

# engine routing (chunk indices 0..15 main, 16 = tail)
GP_SUB = set()                  # gpsimd sub chunks (hurts DVE; keep empty)
PE_SUB = {1, 2, 3, 4, 5, 6, 7, 8, 9, 10}  # chunks subtracted on PE
DVE_SQ = {12, 13, 14, 15, 16}   # square chunks on DVE (rest on ACT)
WARM_MM = 5                     # PE warmup matmuls (512 cols each)
DMA_GROUPS = [1, 1, 2, 2, 2, 3, 3, 3]  # chunk DMAs batched to cut issue cost
PE_FILL = 0                     # dummy matmuls after each chunk's windows (keeps HAM hot)


def _routing():
    gp = GP_SUB
    pe = PE_SUB
    dq = DVE_SQ
    warm = WARM_MM
    if os.environ.get("K_GP_SUB") is not None:
        gp = {int(x) for x in os.environ["K_GP_SUB"].split(",") if x != ""}
    if os.environ.get("K_PE_SUB") is not None:
        pe = {int(x) for x in os.environ["K_PE_SUB"].split(",") if x != ""}
    if os.environ.get("K_DVE_SQ") is not None:
        dq = {int(x) for x in os.environ["K_DVE_SQ"].split(",") if x != ""}
    if os.environ.get("K_WARM") is not None:
        warm = int(os.environ["K_WARM"])
    fill = PE_FILL
    if os.environ.get("K_FILL") is not None:
        fill = int(os.environ["K_FILL"])
    return gp, pe, dq, warm, fill


_program = None


def _build_program():
    try:
        import concourse.bass as bass  # noqa: F401
    except ImportError:
        sys.path.insert(0, "/opt/trn_rl_repo")
        import concourse.bass as bass  # noqa: F401
    from concourse import bacc, mybir, tile
    from concourse.tile import add_dep_helper

    f32 = mybir.dt.float32
    bf16 = mybir.dt.bfloat16
    f8 = mybir.dt.float8e4
    AF = mybir.ActivationFunctionType
    OP = mybir.AluOpType
    AX = mybir.AxisListType

    gp_sub, pe_sub, dve_sq, warm_mm, pe_fill = _routing()
    assert 16 not in pe_sub, "tail chunk cannot go through the PE-sub path"

    nc = bacc.Bacc("TRN2", target_bir_lowering=False, debug=False,
                   num_devices=N_CORES)

    data8 = nc.dram_tensor("data8", (P, F8), f8, kind="ExternalInput")
    aux16 = nc.dram_tensor("aux16", (P, F16), bf16, kind="ExternalInput")
    out = nc.dram_tensor("out", (1, 1), f32, kind="ExternalOutput")

    with tile.TileContext(nc) as tc:
        with tc.tile_pool(name="aux", bufs=1) as apool, \
             tc.tile_pool(name="attr", bufs=1) as tpool, \
             tc.tile_pool(name="stream", bufs=1) as spool, \
             tc.tile_pool(name="diff", bufs=6) as dpool, \
             tc.tile_pool(name="sq", bufs=6) as qpool, \
             tc.tile_pool(name="wrk", bufs=1) as wpool, \
             tc.tile_pool(name="pscol", bufs=1, space="PSUM") as pcol, \
             tc.tile_pool(name="psw", bufs=1, space="PSUM") as psw, \
             tc.tile_pool(name="psd", bufs=2, space="PSUM") as psd, \
             tc.tile_pool(name="psf", bufs=1, space="PSUM") as psf:

            dve_ops = []
            act_ops = []
            pe_ops = []
            gp_ops = []

            # ---------------- PE warmup (no DMA dependency)
            wtile = wpool.tile([P, 512], bf16)
            gp_ops.append(nc.gpsimd.memset(wtile[:], 1.0))
            ones32 = wpool.tile([32, 1], f32)
            gp_ops.append(nc.gpsimd.memset(ones32[:], 1.0))
            wps_warm = psw.tile([32, 512], f32, tag="wwarm")
            if warm_mm:
                for i in range(warm_mm):
                    pe_ops.append(nc.tensor.matmul(
                        wps_warm[:], wtile[:, 0:32], wtile[:],
                        start=True, stop=True))

            # ---------------- DMA: aux, attr+asub, then the 17 chunk pairs
            aux = apool.tile([P, F16], bf16)
            attr = tpool.tile([P, ATTR_LEN + 64], f8)
            dmas = [nc.sync.dma_start(aux[:], aux16.ap())]
            dmas.append(nc.sync.dma_start(attr[:],
                                          data8.ap()[:, 0:ATTR_LEN + 64]))
            chtiles = []
            c0 = 0
            for gi, gsize in enumerate(DMA_GROUPS):
                t = spool.tile([P, gsize * 2 * CHW], f8, tag=f"g{gi}")
                for k in range(gsize):
                    chtiles.append(t[:, k * 2 * CHW:(k + 1) * 2 * CHW])
                off = CH_OFF + c0 * 2 * CHW
                dmas.append(nc.sync.dma_start(
                    t[:], data8.ap()[:, off:off + gsize * 2 * CHW]))
                c0 += gsize
            assert c0 == NCH + 1
            for i in range(len(dmas) - 1):
                add_dep_helper(dmas[i + 1].ins, dmas[i].ins, sync=False,
                               reason="DMA issue order")

            ea_sb = aux[:, EA_OFF:EA_OFF + 384]
            anum_sb = aux[:, ANUM_OFF:ANUM_OFF + 6]
            awin = aux[:, AWIN_OFF:AWIN_OFF + 63]
            zt = aux[:, ZT_OFF:ZT_OFF + 48]
            asub = attr[:, ATTR_LEN:ATTR_LEN + 64]

            # ---------------- weight chain pieces (interleaved into DVE)
            a_sb = wpool.tile([P, 6], f32)
            tot = wpool.tile([P, 1], f32)
            rec = wpool.tile([P, 6], f32)
            ivb = wpool.tile([P, 6], f32)
            aw0 = wpool.tile([P, 128], bf16)
            aw1 = wpool.tile([P, 128], bf16)
            sinh = wpool.tile([P, 384], bf16)
            ssq = wpool.tile([P, 384], bf16)
            ang = wpool.tile([P, 128], bf16)
            w_nat = wpool.tile([P, 128], bf16)

            def w_iv():
                dve_ops.append(nc.vector.tensor_copy(a_sb[:], anum_sb))
                dve_ops.append(nc.vector.tensor_reduce(
                    tot[:], a_sb[:], axis=AX.X, op=OP.add))
                dve_ops.append(nc.vector.reciprocal(rec[:], a_sb[:]))
                dve_ops.append(nc.vector.tensor_scalar_mul(
                    ivb[:], rec[:], tot[:, 0:1]))

            def w_attr():
                dve_ops.append(nc.vector.tensor_scalar_mul(
                    aw0[:], attr[:, 0:128], ivb[:, 0:1]))
                cur, nxt = aw0, aw1
                for j in range(1, 6):
                    dve_ops.append(nc.vector.scalar_tensor_tensor(
                        nxt[:], attr[:, j * 128:(j + 1) * 128],
                        ivb[:, j:j + 1], cur[:], op0=OP.mult, op1=OP.add))
                    cur, nxt = nxt, cur
                return cur

            def w_angle(awt):
                dve_ops.append(nc.vector.tensor_add(
                    ang[:], ssq[:, 0:128], ssq[:, 128:256]))
                dve_ops.append(nc.vector.scalar_tensor_tensor(
                    ang[:], ssq[:, 256:384], 1.0, ang[:],
                    op0=OP.bypass, op1=OP.add))
                dve_ops.append(nc.vector.scalar_tensor_tensor(
                    w_nat[:], ang[:], 2.0, awt[:], op0=OP.mult, op1=OP.mult))

            act_ops.append(nc.scalar.activation(sinh[:], ea_sb, AF.Sin,
                                                bias=0.0, scale=0.5))
            act_ops.append(nc.scalar.activation(ssq[:], sinh[:], AF.Square))

            # ---------------- main loop
            colsq = pcol.tile([32, 512], f32)
            wps = psw.tile([32, 512], f32, tag="wsel")
            w_sb = wpool.tile([32, 512], f32)
            n_mm = 2 * (NCH + 1)
            mm_idx = [0]
            awt = [None]
            w_copy_pending = [True]
            pending_wins = []

            def emit_win(lhsT, mv):
                pe_ops.append(nc.tensor.matmul(
                    colsq[:], lhsT, mv,
                    start=(mm_idx[0] == 0), stop=(mm_idx[0] == n_mm - 1)))
                mm_idx[0] += 1

            def flush_wins():
                for lhsT, mv in pending_wins:
                    emit_win(lhsT, mv)
                pending_wins.clear()

            def emit_selectors():
                for beta in range(4):
                    sel = aux[:, ASEL_OFF + 32 * beta:
                              ASEL_OFF + 32 * beta + 32]
                    pe_ops.append(nc.tensor.matmul(
                        wps[:, 128 * beta:128 * beta + 128], sel, w_nat[:],
                        start=True, stop=True))

            for c in range(NCH + 1):
                t = chtiles[c]
                if c in pe_sub:
                    # PE subtract: pair-interleaved quadrants -> psum diff
                    pdiff = psd.tile([P, CHW], f32, tag="pd")
                    for h in range(2):
                        for half in range(2):
                            mvo = (2 * h + half) * 512
                            pe_ops.append(nc.tensor.matmul(
                                pdiff[64 * half:64 * half + 64,
                                      h * 512:(h + 1) * 512],
                                asub, t[:, mvo:mvo + 512],
                                start=True, stop=True))
                    flush_wins()
                    sq = qpool.tile([P, CHW], bf16, tag="sq")
                    act_ops.append(nc.scalar.activation(
                        sq[:], pdiff[:], AF.Square))
                    for h in range(2):
                        r = 2 * c + h
                        pending_wins.append((awin[:, 31 - r:63 - r],
                                             sq[:, h * 512:(h + 1) * 512]))
                else:
                    df = dpool.tile([P, CHW], bf16, tag="diff")
                    if c in gp_sub:
                        gp_ops.append(nc.gpsimd.tensor_sub(
                            df[:], t[:, 0:CHW], t[:, CHW:2 * CHW]))
                    else:
                        dve_ops.append(nc.vector.tensor_sub(
                            df[:], t[:, 0:CHW], t[:, CHW:2 * CHW]))
                    sq = qpool.tile([P, CHW], bf16, tag="sq")
                    if c in dve_sq:
                        dve_ops.append(nc.vector.tensor_mul(
                            sq[:], df[:], df[:]))
                    else:
                        act_ops.append(nc.scalar.activation(
                            sq[:], df[:], AF.Square))
                    flush_wins()
                    for h in range(2):
                        mv = sq[:, h * 512:(h + 1) * 512]
                        if c < NCH:
                            r = 2 * c + h
                            pending_wins.append((awin[:, 31 - r:63 - r], mv))
                        else:
                            g = h
                            pending_wins.append(
                                (zt[:, 16 - 16 * g:48 - 16 * g], mv))
                for _ in range(pe_fill):
                    pe_ops.append(nc.tensor.matmul(
                        wps_warm[:], wtile[:, 0:32], wtile[:],
                        start=True, stop=True))
                # weave the weight chain between early chunks
                if c == 0:
                    w_iv()
                elif c == 1:
                    awt[0] = w_attr()
                elif c == 2:
                    w_angle(awt[0])
                elif c == 4:
                    emit_selectors()
                elif c == 5:
                    act_ops.append(nc.scalar.activation(
                        w_sb[:], wps[:], AF.Copy))
                    w_copy_pending[0] = False
            flush_wins()
            assert not w_copy_pending[0]
            assert mm_idx[0] == n_mm

            # ---------------- final weighted reduce
            scr = wpool.tile([32, 512], f32)
            part = wpool.tile([32, 1], f32)
            dve_ops.append(nc.vector.scalar_tensor_tensor(
                scr[:], colsq[:], 1.0, w_sb[:], op0=OP.bypass, op1=OP.mult,
                accum_out=part[:]))
            fin = psf.tile([1, 1], f32)
            pe_ops.append(nc.tensor.matmul(fin[:], ones32[:], part[:],
                                           start=True, stop=True))
            res = wpool.tile([1, 1], f32)
            dve_ops.append(nc.vector.tensor_copy(res[:], fin[:]))
            nc.sync.dma_start(out.ap(), res[:])

            # ---------------- per-engine issue-order pins
            for ops in (dve_ops, act_ops, pe_ops, gp_ops):
                for i in range(len(ops) - 1):
                    add_dep_helper(ops[i + 1].ins, ops[i].ins, sync=False,
                                   reason="engine order")

    nc.compile()
    return nc


def _get_program():
    global _program
    if _program is None:
        _program = _build_program()
    return _program


def _make_in_maps(inp, label, ea, attribute, attribute_num):
    import ml_dtypes
    e4 = ml_dtypes.float8_e4m3
    bf = ml_dtypes.bfloat16
    _, pe_sub, _, _, _ = _routing()
    inp = np.asarray(inp, dtype=np.float32)
    label = np.asarray(label, dtype=np.float32)
    ea = np.asarray(ea, dtype=np.float32)
    attribute = np.asarray(attribute, dtype=np.int32)
    anum_row = np.asarray(attribute_num, dtype=np.float32).reshape(6)

    # constant fields (same for all cores)
    awin = np.zeros((P, 63), dtype=np.float32)
    awin[:, 31] = 1.0
    ztm = np.zeros((P, 48), dtype=np.float32)
    for k in range(16):
        ztm[8 * k:8 * k + 8, 16 + k] = 1.0
    asel = np.zeros((P, 128), dtype=np.float32)
    for beta in range(4):
        for r in range(32):
            asel[4 * r + beta, 32 * beta + r] = 1.0
    asub = np.zeros((P, 64), dtype=np.float32)
    for k in range(64):
        asub[2 * k, k] = 1.0
        asub[2 * k + 1, k] = -1.0

    in_maps = []
    for c in range(N_CORES):
        sl = slice(c * BS, (c + 1) * BS)
        xi = inp[:, sl]
        xl = label[:, sl]
        d8 = np.empty((P, F8), dtype=e4)
        d8[:, ATTR_OFF:ATTR_OFF + ATTR_LEN] = (
            attribute[:, sl].reshape(6, P, 128).transpose(1, 0, 2)
            .reshape(P, ATTR_LEN).astype(e4))
        d8[:, ASUB_OFF:ASUB_OFF + 64] = asub.astype(e4)
        mi = xi[0:128].astype(e4)
        ml = xl[0:128].astype(e4)
        for k in range(NCH):
            off = CH_OFF + k * 2 * CHW
            if k in pe_sub:
                # pair-interleave: halves of 512 cols, partition 2j = inp
                # d-row j(+64), 2j+1 = label d-row j(+64)
                blk = np.empty((P, 2 * CHW), dtype=e4)
                for h in range(2):
                    for half in range(2):
                        src_i = mi[64 * half:64 * half + 64,
                                   k * CHW + h * 512:k * CHW + (h + 1) * 512]
                        src_l = ml[64 * half:64 * half + 64,
                                   k * CHW + h * 512:k * CHW + (h + 1) * 512]
                        pi = np.empty((128, 512), dtype=e4)
                        pi[0::2] = src_i
                        pi[1::2] = src_l
                        blk[:, (2 * h + half) * 512:
                            (2 * h + half + 1) * 512] = pi
                d8[:, off:off + 2 * CHW] = blk
            else:
                d8[:, off:off + CHW] = mi[:, k * CHW:(k + 1) * CHW]
                d8[:, off + CHW:off + 2 * CHW] = ml[:, k * CHW:(k + 1) * CHW]
        toff = CH_OFF + NCH * 2 * CHW
        ti = (xi[128:136].reshape(8, 2, 16, 512).transpose(2, 0, 1, 3)
              .reshape(P, NTAIL).astype(e4))
        tl = (xl[128:136].reshape(8, 2, 16, 512).transpose(2, 0, 1, 3)
              .reshape(P, NTAIL).astype(e4))
        d8[:, toff:toff + NTAIL] = ti
        d8[:, toff + NTAIL:toff + 2 * NTAIL] = tl

        a16 = np.zeros((P, F16), dtype=bf)
        a16[:, EA_OFF:EA_OFF + 384] = (
            ea[:, sl].reshape(3, P, 128).transpose(1, 0, 2)
            .reshape(P, 384).astype(bf))
        a16[:, ANUM_OFF:ANUM_OFF + 6] = anum_row.astype(bf)
        a16[:, AWIN_OFF:AWIN_OFF + 63] = awin.astype(bf)
        a16[:, ZT_OFF:ZT_OFF + 48] = ztm.astype(bf)
        a16[:, ASEL_OFF:ASEL_OFF + 128] = asel.astype(bf)
        in_maps.append({"data8": d8, "aux16": a16})
    return in_maps


def run(inputs, trace=False, trace_cores=None):
    """Run on hardware; returns (result_scalar, BassKernelResults)."""
    try:
        from concourse.bass_utils import run_bass_kernel_spmd
    except ImportError:
        sys.path.insert(0, "/opt/trn_rl_repo")
        from concourse.bass_utils import run_bass_kernel_spmd
    nc = _get_program()
    in_maps = _make_in_maps(**inputs)
    kwargs = {}
    if trace:
        kwargs["trace"] = True
        if trace_cores is not None:
            kwargs["trace_cores"] = trace_cores
    res = run_bass_kernel_spmd(nc, in_maps, core_ids=list(range(N_CORES)),
                               **kwargs)
    total = 0.0
    for r in res.results:
        total += float(r["out"].astype(np.float64).sum())
    value = np.asarray(total / (D * B), dtype=np.float32)
    return value, res


def kernel(**inputs):
    value, _ = run(inputs)
    return value


# revision 11
# speedup vs baseline: 1.4520x; 1.0303x over previous
"""Weighted-MSE loss kernel (nn_LossWithEuler) for 8 Trainium2 NeuronCores.

loss = mean(weight[b] * (inp[d,b] - label[d,b])^2)
  weight[b]  = attr_w[b] * angle_w[b]
  attr_w[b]  = sum_j (attribute[j,b]==1) * (sum(attribute_num)/attribute_num[j])
  angle_w[b] = sum_j (1 - cos(ea[j,b])) = sum_j 2*sin(ea[j,b]/2)^2

Sharding: batch axis B=131072 split across 8 cores (16384 b's each).

inp/label are host-cast to fp8-e4m3 (quantization error on the final mean
~1.6e-3 vs the 2e-2 gate; halves the HBM stream vs bf16) and delivered in
COLUMN layout: partition = d (rows 0..127), free = b.  The leftover 8
d-rows (128..135) are repacked into 1024 "tail" columns of 16 b-blocks
(8 d's each).  In this layout the d-reduction is a ones-matmul on the
otherwise-idle TensorEngine: 32 sliding-window A-matmuls (512 cols each)
+ 2 block-A tail matmuls accumulate sum_d diff^2 per b into ONE [32,512]
f32 PSUM tile (colsq[r,q] = b = r*512+q).  A PE warmup spin at kernel
start keeps the PE at its 2.4 GHz HAM clock.  Chunks routed to PE_SUB are
delivered pair-interleaved (partition 2k = inp d-row, 2k+1 = label d-row)
and subtracted BY the PE with a fixed +-1 stationary; ACT squares those
from PSUM.  Remaining chunks: DVE subtracts (fp8 TT runs 1x), ACT (+DVE
for the last chunks) squares.  GPSIMD is kept idle: concurrent GPSIMD
tensor ops slow DVE ops ~2.5x (SBUF contention, measured).  The per-b
weight is computed in the natural [128,128] layout (b = p*128+f) and
shuffled to the colsq layout by 4 PE selector matmuls (w'[r,128*beta+f] =
w_nat[4r+beta, f]).  Final: one fused STT (colsq * w' with free-dim
accumulate), a [32]->[1] ones matmul, one f32 DMA out per core; host sums
8 partials / (D*B).
"""

import os
import sys
import numpy as np

D = 136
B = 131072
N_CORES = 8
BS = B // N_CORES       # 16384 b's per core
P = 128
NTAIL = BS // 16        # 1024 tail columns (16 b's x 8 d's each)
NCH = 16                # main chunks of 1024 cols
CHW = 1024              # chunk width (2 PE windows of 512)

# data8 field offsets (fp8 bytes per partition)
ATTR_OFF = 0
ATTR_LEN = 6 * 128      # 768
ASUB_OFF = ATTR_OFF + ATTR_LEN          # [128, 64] +-1 pair-subtract matrix
CH_OFF = ASUB_OFF + 64                  # chunks: 17 x (i 1024 | l 1024)
F8 = CH_OFF + (NCH + 1) * 2 * CHW

# aux16 field offsets (bf16 elems per partition)
EA_OFF = 0              # [128, 384]
ANUM_OFF = 384          # [128, 6]
AWIN_OFF = 392          # [128, 63]  main sliding ones-window
ZT_OFF = 456            # [128, 48]  tail block window
ASEL_OFF = 504          # [128, 4*32] selector matrices
F16 = 640

# engine routing (chunk indices 0..15 main, 16 = tail)
GP_SUB = set()                  # gpsimd sub chunks (hurts DVE; keep empty)
PE_SUB = {1, 2, 3, 4, 5, 6, 7, 8, 9}  # chunks subtracted on PE
DVE_SQ = {13, 14, 15, 16}       # square chunks on DVE (rest on ACT)
WARM_MM = 5                     # PE warmup matmuls (512 cols each)
DMA_GROUPS = [1, 1, 2, 2, 2, 3, 3, 3]  # chunk DMAs batched to cut issue cost
PE_FILL = 0                     # dummy matmuls after each chunk's windows (keeps HAM hot)


def _routing():
    gp = GP_SUB
    pe = PE_SUB
    dq = DVE_SQ
    warm = WARM_MM
    if os.environ.get("K_GP_SUB") is not None:
        gp = {int(x) for x in os.environ["K_GP_SUB"].split(",") if x != ""}
    if os.environ.get("K_PE_SUB") is not None:
        pe = {int(x) for x in os.environ["K_PE_SUB"].split(",") if x != ""}
    if os.environ.get("K_DVE_SQ") is not None:
        dq = {int(x) for x in os.environ["K_DVE_SQ"].split(",") if x != ""}
    if os.environ.get("K_WARM") is not None:
        warm = int(os.environ["K_WARM"])
    fill = PE_FILL
    if os.environ.get("K_FILL") is not None:
        fill = int(os.environ["K_FILL"])
    return gp, pe, dq, warm, fill


_program = None


def _build_program():
    try:
        import concourse.bass as bass  # noqa: F401
    except ImportError:
        sys.path.insert(0, "/opt/trn_rl_repo")
        import concourse.bass as bass  # noqa: F401
    from concourse import bacc, mybir, tile
    from concourse.tile import add_dep_helper

    f32 = mybir.dt.float32
    bf16 = mybir.dt.bfloat16
    f8 = mybir.dt.float8e4
    AF = mybir.ActivationFunctionType
    OP = mybir.AluOpType
    AX = mybir.AxisListType

    gp_sub, pe_sub, dve_sq, warm_mm, pe_fill = _routing()
    assert 16 not in pe_sub, "tail chunk cannot go through the PE-sub path"

    nc = bacc.Bacc("TRN2", target_bir_lowering=False, debug=False,
                   num_devices=N_CORES)

    data8 = nc.dram_tensor("data8", (P, F8), f8, kind="ExternalInput")
    aux16 = nc.dram_tensor("aux16", (P, F16), bf16, kind="ExternalInput")
    out = nc.dram_tensor("out", (1, 1), f32, kind="ExternalOutput")

    with tile.TileContext(nc) as tc:
        with tc.tile_pool(name="aux", bufs=1) as apool, \
             tc.tile_pool(name="attr", bufs=1) as tpool, \
             tc.tile_pool(name="stream", bufs=1) as spool, \
             tc.tile_pool(name="diff", bufs=6) as dpool, \
             tc.tile_pool(name="sq", bufs=6) as qpool, \
             tc.tile_pool(name="wrk", bufs=1) as wpool, \
             tc.tile_pool(name="pscol", bufs=1, space="PSUM") as pcol, \
             tc.tile_pool(name="psw", bufs=1, space="PSUM") as psw, \
             tc.tile_pool(name="psd", bufs=2, space="PSUM") as psd, \
             tc.tile_pool(name="psf", bufs=1, space="PSUM") as psf:

            dve_ops = []
            act_ops = []
            pe_ops = []
            gp_ops = []

            # ---------------- PE warmup (no DMA dependency)
            wtile = wpool.tile([P, 512], bf16)
            gp_ops.append(nc.gpsimd.memset(wtile[:], 1.0))
            ones32 = wpool.tile([32, 1], f32)
            gp_ops.append(nc.gpsimd.memset(ones32[:], 1.0))
            wps_warm = psw.tile([32, 512], f32, tag="wwarm")
            if warm_mm:
                for i in range(warm_mm):
                    pe_ops.append(nc.tensor.matmul(
                        wps_warm[:], wtile[:, 0:32], wtile[:],
                        start=True, stop=True))

            # ---------------- DMA: aux, attr+asub, then the 17 chunk pairs
            aux = apool.tile([P, F16], bf16)
            attr = tpool.tile([P, ATTR_LEN + 64], f8)
            dmas = []
            chtiles = []
            c0 = 0
            for gi, gsize in enumerate(DMA_GROUPS):
                t = spool.tile([P, gsize * 2 * CHW], f8, tag=f"g{gi}")
                for k in range(gsize):
                    chtiles.append(t[:, k * 2 * CHW:(k + 1) * 2 * CHW])
                off = CH_OFF + c0 * 2 * CHW
                dmas.append(nc.sync.dma_start(
                    t[:], data8.ap()[:, off:off + gsize * 2 * CHW]))
                c0 += gsize
                if gi == 0:
                    dmas.append(nc.sync.dma_start(aux[:], aux16.ap()))
                    dmas.append(nc.sync.dma_start(
                        attr[:], data8.ap()[:, 0:ATTR_LEN + 64]))
            assert c0 == NCH + 1
            for i in range(len(dmas) - 1):
                add_dep_helper(dmas[i + 1].ins, dmas[i].ins, sync=False,
                               reason="DMA issue order")

            ea_sb = aux[:, EA_OFF:EA_OFF + 384]
            anum_sb = aux[:, ANUM_OFF:ANUM_OFF + 6]
            awin = aux[:, AWIN_OFF:AWIN_OFF + 63]
            zt = aux[:, ZT_OFF:ZT_OFF + 48]
            asub = attr[:, ATTR_LEN:ATTR_LEN + 64]

            # ---------------- weight chain pieces (interleaved into DVE)
            a_sb = wpool.tile([P, 6], f32)
            tot = wpool.tile([P, 1], f32)
            rec = wpool.tile([P, 6], f32)
            ivb = wpool.tile([P, 6], f32)
            aw0 = wpool.tile([P, 128], bf16)
            aw1 = wpool.tile([P, 128], bf16)
            sinh = wpool.tile([P, 384], bf16)
            ssq = wpool.tile([P, 384], bf16)
            ang = wpool.tile([P, 128], bf16)
            w_nat = wpool.tile([P, 128], bf16)

            def w_iv():
                dve_ops.append(nc.vector.tensor_copy(a_sb[:], anum_sb))
                dve_ops.append(nc.vector.tensor_reduce(
                    tot[:], a_sb[:], axis=AX.X, op=OP.add))
                dve_ops.append(nc.vector.reciprocal(rec[:], a_sb[:]))
                dve_ops.append(nc.vector.tensor_scalar_mul(
                    ivb[:], rec[:], tot[:, 0:1]))

            def w_attr():
                dve_ops.append(nc.vector.tensor_scalar_mul(
                    aw0[:], attr[:, 0:128], ivb[:, 0:1]))
                cur, nxt = aw0, aw1
                for j in range(1, 6):
                    dve_ops.append(nc.vector.scalar_tensor_tensor(
                        nxt[:], attr[:, j * 128:(j + 1) * 128],
                        ivb[:, j:j + 1], cur[:], op0=OP.mult, op1=OP.add))
                    cur, nxt = nxt, cur
                return cur

            def w_angle(awt):
                dve_ops.append(nc.vector.tensor_add(
                    ang[:], ssq[:, 0:128], ssq[:, 128:256]))
                dve_ops.append(nc.vector.scalar_tensor_tensor(
                    ang[:], ssq[:, 256:384], 1.0, ang[:],
                    op0=OP.bypass, op1=OP.add))
                dve_ops.append(nc.vector.scalar_tensor_tensor(
                    w_nat[:], ang[:], 2.0, awt[:], op0=OP.mult, op1=OP.mult))

            act_ops.append(nc.scalar.activation(sinh[:], ea_sb, AF.Sin,
                                                bias=0.0, scale=0.5))
            act_ops.append(nc.scalar.activation(ssq[:], sinh[:], AF.Square))

            # ---------------- main loop
            colsq = pcol.tile([32, 512], f32)
            wps = psw.tile([32, 512], f32, tag="wsel")
            w_sb = wpool.tile([32, 512], f32)
            n_mm = 2 * (NCH + 1)
            mm_idx = [0]
            awt = [None]
            w_copy_pending = [True]
            pending_wins = []

            def emit_win(lhsT, mv):
                pe_ops.append(nc.tensor.matmul(
                    colsq[:], lhsT, mv,
                    start=(mm_idx[0] == 0), stop=(mm_idx[0] == n_mm - 1)))
                mm_idx[0] += 1

            def flush_wins():
                for lhsT, mv in pending_wins:
                    emit_win(lhsT, mv)
                pending_wins.clear()

            def emit_selectors():
                for beta in range(4):
                    sel = aux[:, ASEL_OFF + 32 * beta:
                              ASEL_OFF + 32 * beta + 32]
                    pe_ops.append(nc.tensor.matmul(
                        wps[:, 128 * beta:128 * beta + 128], sel, w_nat[:],
                        start=True, stop=True))

            for c in range(NCH + 1):
                t = chtiles[c]
                if c in pe_sub:
                    # PE subtract: pair-interleaved quadrants -> psum diff
                    pdiff = psd.tile([P, CHW], f32, tag="pd")
                    for h in range(2):
                        for half in range(2):
                            mvo = (2 * h + half) * 512
                            pe_ops.append(nc.tensor.matmul(
                                pdiff[64 * half:64 * half + 64,
                                      h * 512:(h + 1) * 512],
                                asub, t[:, mvo:mvo + 512],
                                start=True, stop=True))
                    flush_wins()
                    sq = qpool.tile([P, CHW], bf16, tag="sq")
                    act_ops.append(nc.scalar.activation(
                        sq[:], pdiff[:], AF.Square))
                    for h in range(2):
                        r = 2 * c + h
                        pending_wins.append((awin[:, 31 - r:63 - r],
                                             sq[:, h * 512:(h + 1) * 512]))
                else:
                    df = dpool.tile([P, CHW], bf16, tag="diff")
                    if c in gp_sub:
                        gp_ops.append(nc.gpsimd.tensor_sub(
                            df[:], t[:, 0:CHW], t[:, CHW:2 * CHW]))
                    else:
                        dve_ops.append(nc.vector.tensor_sub(
                            df[:], t[:, 0:CHW], t[:, CHW:2 * CHW]))
                    sq = qpool.tile([P, CHW], bf16, tag="sq")
                    if c in dve_sq:
                        dve_ops.append(nc.vector.tensor_mul(
                            sq[:], df[:], df[:]))
                    else:
                        act_ops.append(nc.scalar.activation(
                            sq[:], df[:], AF.Square))
                    flush_wins()
                    for h in range(2):
                        mv = sq[:, h * 512:(h + 1) * 512]
                        if c < NCH:
                            r = 2 * c + h
                            pending_wins.append((awin[:, 31 - r:63 - r], mv))
                        else:
                            g = h
                            pending_wins.append(
                                (zt[:, 16 - 16 * g:48 - 16 * g], mv))
                for _ in range(pe_fill):
                    pe_ops.append(nc.tensor.matmul(
                        wps_warm[:], wtile[:, 0:32], wtile[:],
                        start=True, stop=True))
                # weave the weight chain between early chunks
                if c == 0:
                    w_iv()
                elif c == 1:
                    awt[0] = w_attr()
                elif c == 2:
                    w_angle(awt[0])
                elif c == 4:
                    emit_selectors()
                elif c == 5:
                    act_ops.append(nc.scalar.activation(
                        w_sb[:], wps[:], AF.Copy))
                    w_copy_pending[0] = False
            flush_wins()
            assert not w_copy_pending[0]
            assert mm_idx[0] == n_mm

            # ---------------- final weighted reduce
            scr = wpool.tile([32, 512], f32)
            part = wpool.tile([32, 1], f32)
            dve_ops.append(nc.vector.scalar_tensor_tensor(
                scr[:], colsq[:], 1.0, w_sb[:], op0=OP.bypass, op1=OP.mult,
                accum_out=part[:]))
            fin = psf.tile([1, 1], f32)
            pe_ops.append(nc.tensor.matmul(fin[:], ones32[:], part[:],
                                           start=True, stop=True))
            res = wpool.tile([1, 1], f32)
            dve_ops.append(nc.vector.tensor_copy(res[:], fin[:]))
            nc.sync.dma_start(out.ap(), res[:])

            # ---------------- per-engine issue-order pins
            for ops in (dve_ops, act_ops, pe_ops, gp_ops):
                for i in range(len(ops) - 1):
                    add_dep_helper(ops[i + 1].ins, ops[i].ins, sync=False,
                                   reason="engine order")

    nc.compile()
    return nc


def _get_program():
    global _program
    if _program is None:
        _program = _build_program()
    return _program


def _make_in_maps(inp, label, ea, attribute, attribute_num):
    import ml_dtypes
    e4 = ml_dtypes.float8_e4m3
    bf = ml_dtypes.bfloat16
    _, pe_sub, _, _, _ = _routing()
    inp = np.asarray(inp, dtype=np.float32)
    label = np.asarray(label, dtype=np.float32)
    ea = np.asarray(ea, dtype=np.float32)
    attribute = np.asarray(attribute, dtype=np.int32)
    anum_row = np.asarray(attribute_num, dtype=np.float32).reshape(6)

    # constant fields (same for all cores)
    awin = np.zeros((P, 63), dtype=np.float32)
    awin[:, 31] = 1.0
    ztm = np.zeros((P, 48), dtype=np.float32)
    for k in range(16):
        ztm[8 * k:8 * k + 8, 16 + k] = 1.0
    asel = np.zeros((P, 128), dtype=np.float32)
    for beta in range(4):
        for r in range(32):
            asel[4 * r + beta, 32 * beta + r] = 1.0
    asub = np.zeros((P, 64), dtype=np.float32)
    for k in range(64):
        asub[2 * k, k] = 1.0
        asub[2 * k + 1, k] = -1.0

    in_maps = []
    for c in range(N_CORES):
        sl = slice(c * BS, (c + 1) * BS)
        xi = inp[:, sl]
        xl = label[:, sl]
        d8 = np.empty((P, F8), dtype=e4)
        d8[:, ATTR_OFF:ATTR_OFF + ATTR_LEN] = (
            attribute[:, sl].reshape(6, P, 128).transpose(1, 0, 2)
            .reshape(P, ATTR_LEN).astype(e4))
        d8[:, ASUB_OFF:ASUB_OFF + 64] = asub.astype(e4)
        mi = xi[0:128].astype(e4)
        ml = xl[0:128].astype(e4)
        for k in range(NCH):
            off = CH_OFF + k * 2 * CHW
            if k in pe_sub:
                # pair-interleave: halves of 512 cols, partition 2j = inp
                # d-row j(+64), 2j+1 = label d-row j(+64)
                blk = np.empty((P, 2 * CHW), dtype=e4)
                for h in range(2):
                    for half in range(2):
                        src_i = mi[64 * half:64 * half + 64,
                                   k * CHW + h * 512:k * CHW + (h + 1) * 512]
                        src_l = ml[64 * half:64 * half + 64,
                                   k * CHW + h * 512:k * CHW + (h + 1) * 512]
                        pi = np.empty((128, 512), dtype=e4)
                        pi[0::2] = src_i
                        pi[1::2] = src_l
                        blk[:, (2 * h + half) * 512:
                            (2 * h + half + 1) * 512] = pi
                d8[:, off:off + 2 * CHW] = blk
            else:
                d8[:, off:off + CHW] = mi[:, k * CHW:(k + 1) * CHW]
                d8[:, off + CHW:off + 2 * CHW] = ml[:, k * CHW:(k + 1) * CHW]
        toff = CH_OFF + NCH * 2 * CHW
        ti = (xi[128:136].reshape(8, 2, 16, 512).transpose(2, 0, 1, 3)
              .reshape(P, NTAIL).astype(e4))
        tl = (xl[128:136].reshape(8, 2, 16, 512).transpose(2, 0, 1, 3)
              .reshape(P, NTAIL).astype(e4))
        d8[:, toff:toff + NTAIL] = ti
        d8[:, toff + NTAIL:toff + 2 * NTAIL] = tl

        a16 = np.zeros((P, F16), dtype=bf)
        a16[:, EA_OFF:EA_OFF + 384] = (
            ea[:, sl].reshape(3, P, 128).transpose(1, 0, 2)
            .reshape(P, 384).astype(bf))
        a16[:, ANUM_OFF:ANUM_OFF + 6] = anum_row.astype(bf)
        a16[:, AWIN_OFF:AWIN_OFF + 63] = awin.astype(bf)
        a16[:, ZT_OFF:ZT_OFF + 48] = ztm.astype(bf)
        a16[:, ASEL_OFF:ASEL_OFF + 128] = asel.astype(bf)
        in_maps.append({"data8": d8, "aux16": a16})
    return in_maps


def run(inputs, trace=False, trace_cores=None):
    """Run on hardware; returns (result_scalar, BassKernelResults)."""
    try:
        from concourse.bass_utils import run_bass_kernel_spmd
    except ImportError:
        sys.path.insert(0, "/opt/trn_rl_repo")
        from concourse.bass_utils import run_bass_kernel_spmd
    nc = _get_program()
    in_maps = _make_in_maps(**inputs)
    kwargs = {}
    if trace:
        kwargs["trace"] = True
        if trace_cores is not None:
            kwargs["trace_cores"] = trace_cores
    res = run_bass_kernel_spmd(nc, in_maps, core_ids=list(range(N_CORES)),
                               **kwargs)
    total = 0.0
    for r in res.results:
        total += float(r["out"].astype(np.float64).sum())
    value = np.asarray(total / (D * B), dtype=np.float32)
    return value, res


def kernel(**inputs):
    value, _ = run(inputs)
    return value
